# revision 25
# baseline (speedup 1.0000x reference)
"""Wilson-clover Dirac operator D_WC on Trainium2, 8-core SPMD.

Self-contained: hardcodes LAT=(32,16,16,16), shards the T axis across 8
cores with host-side halo slices (t0-1..t0+4), computes everything
site-locally on-device with DVE elementwise math in an SoA layout:

  plane[p, col]: p = z*8 + (y>>1),  col = plane_idx*W + (s-base)*32
                 + (y&1)*16 + x          (s = local T-slice, W = ns*32)

Shifts: T = column offset (free), X = 2-piece AP copy, Z = 2-piece
partition-offset DMA, Y = 1 copy + 2 DMAs.
"""
import numpy as np

# ----------------------------------------------------------------------
# constants (physics)
# ----------------------------------------------------------------------
MASS, C_SW = 0.5, 1.0
_i = 1j
G1 = np.array([[0,0,0,_i],[0,0,_i,0],[0,-_i,0,0],[-_i,0,0,0]], dtype=np.complex64)
G2 = np.array([[0,0,0,-1],[0,0,1,0],[0,1,0,0],[-1,0,0,0]], dtype=np.complex64)
G3 = np.array([[0,0,_i,0],[0,0,0,-_i],[-_i,0,0,0],[0,_i,0,0]], dtype=np.complex64)
G4 = np.array([[0,0,1,0],[0,0,0,1],[1,0,0,0],[0,1,0,0]], dtype=np.complex64)
GAMMA = [G1, G2, G3, G4]
I4 = np.eye(4, dtype=np.complex64)
SIGMA = [[(0.5j * (GAMMA[m] @ GAMMA[n] - GAMMA[n] @ GAMMA[m])).astype(np.complex64)
          for n in range(4)] for m in range(4)]

T_, Z_, Y_, X_ = 32, 16, 16, 16
NCORES, TLOC, NSH = 8, 4, 6       # halo slices per core
SL = 32                           # cols per slice
ENGINE_MIX = True                 # offload product row i=2 to GPSIMD, copies to ACT
F32 = None                        # set after mybir import (device build only)

# ----------------------------------------------------------------------
# spin-structure extraction (numerical)
# ----------------------------------------------------------------------
def _col_struct(M):
    """M has single-nonzero columns: return per-col (row, phase)."""
    out = []
    for s in range(4):
        col = M[:, s]
        r = int(np.argmax(np.abs(col)))
        ph = complex(col[r])
        assert np.sum(np.abs(col) > 1e-6) == 1, (M, s)
        out.append((r, ph))
    return out

def _proj_struct(P):
    """P rank-2 with cols 2,3 = rho * cols q in {0,1}. Returns (q2,rho2,q3,rho3)."""
    res = []
    for s in (2, 3):
        found = None
        for j in (0, 1):
            c, cj = P[:, s], P[:, j]
            nz = np.abs(cj) > 1e-6
            if not nz.any():
                continue
            ratio = c[nz] / cj[nz]
            if np.allclose(ratio, ratio[0], atol=1e-5) and np.allclose(
                    c, ratio[0] * cj, atol=1e-5):
                found = (j, complex(ratio[0]))
                break
        assert found is not None, P
        res.append(found)
    return res

# per direction k: gamma column structure and projector relations
GCOL = [_col_struct(GAMMA[k]) for k in range(4)]           # (p(s), g(s))
PROJ_F = [_proj_struct(GAMMA[k] + I4) for k in range(4)]   # for psi@(G+I)
PROJ_B = [_proj_struct(GAMMA[k] - I4) for k in range(4)]   # for psi@(G-I)
SIGCOL = [[_col_struct(SIGMA[m][n]) if m != n else None for n in range(4)]
          for m in range(4)]

def _phase_parts(ph):
    """phase in {1,-1,i,-i} -> for v = ph*u:
    re(v) = sr*u.[pr]  im(v) = si*u.[pi]   (0=re,1=im planes of u)"""
    if abs(ph - 1) < 1e-5:   return (0, 1.0, 1, 1.0)
    if abs(ph + 1) < 1e-5:   return (0, -1.0, 1, -1.0)
    if abs(ph - 1j) < 1e-5:  return (1, -1.0, 0, 1.0)
    if abs(ph + 1j) < 1e-5:  return (1, 1.0, 0, -1.0)
    raise AssertionError(ph)

# ----------------------------------------------------------------------
# host layout helpers
# ----------------------------------------------------------------------
def _to_planes(vol):
    """vol [ns, Z, Y, X] -> [128, ns*32]"""
    ns = vol.shape[0]
    v = vol.reshape(ns, 16, 8, 2, 16)        # s z yh yl x
    v = np.transpose(v, (1, 2, 0, 3, 4))     # z yh s yl x
    return np.ascontiguousarray(v.reshape(128, ns * 32))

def _from_planes(pl, ns):
    v = pl.reshape(16, 8, ns, 2, 16)
    return np.transpose(v, (2, 0, 1, 3, 4)).reshape(ns, 16, 16, 16)

def _host_inputs(field_re, field_im, gauge_re, gauge_im, core):
    t0 = core * TLOC
    ts = [(t0 - 1 + s) % T_ for s in range(NSH)]
    f = np.stack([field_re[ts], field_im[ts]], axis=0)   # [2,6,Z,Y,X,3,4]
    g = np.stack([gauge_re[:, ts], gauge_im[:, ts]], axis=0)  # [2,4,6,Z,Y,X,3,3]

    # field planes: e=(c*4+sp), plane=e*2+r
    fp = np.empty((24, 128, NSH * SL), np.float32)
    for c in range(3):
        for sp in range(4):
            for r in range(2):
                fp[(c * 4 + sp) * 2 + r] = _to_planes(f[r, :, :, :, :, c, sp])
    fld = np.ascontiguousarray(fp.transpose(1, 0, 2).reshape(128, 24 * NSH * SL))

    # gauge planes: dir k, e=(a*3+b), plane=(k*9+e)*2+r
    gp = np.empty((72, 128, NSH * SL), np.float32)
    for k in range(4):
        for a in range(3):
            for b in range(3):
                for r in range(2):
                    gp[(k * 9 + a * 3 + b) * 2 + r] = _to_planes(
                        g[r, k, :, :, :, :, a, b])
    gg = np.ascontiguousarray(gp.transpose(1, 0, 2).reshape(128, 72 * NSH * SL))
    return {"fld": fld, "gg": gg}

def _host_output(outp_flat):
    """device out [128, 24*4*32] -> [TLOC, Z,Y,X, 3,4,2]"""
    pl = outp_flat.reshape(128, 24, TLOC * SL).transpose(1, 0, 2)
    out = np.empty((TLOC, 16, 16, 16, 3, 4, 2), np.float32)
    for c in range(3):
        for sp in range(4):
            for r in range(2):
                out[..., c, sp, r] = _from_planes(pl[(c * 4 + sp) * 2 + r], TLOC)
    return out

# ----------------------------------------------------------------------
# device program
# ----------------------------------------------------------------------
class Grp:
    """Group of planes in one SBUF tile. nent complex entries (re+im planes).
    base = slice index of col 0; ns slices; W = ns*32 cols per plane."""
    def __init__(self, tile, nent, base, ns):
        self.t, self.nent, self.base, self.ns = tile, nent, base, ns
        self.W = ns * SL

    def fl(self):
        return self.t.rearrange("p (q w) -> p q w", q=self.nent * 2, w=self.W)

    def cs(self, s0, s1):
        return ((s0 - self.base) * SL, (s1 - self.base) * SL)

    def pap(self, e, r, s0, s1):
        """single plane AP [128, cols]"""
        c0, c1 = self.cs(s0, s1)
        v = self.fl()[:, (e * 2 + r):(e * 2 + r + 1), c0:c1]
        return v  # [p,1,w]

    def gap(self, e0, estep, n, r, s0, s1):
        """packed-entry AP [p, n, w]: entries e0 + i*estep, fixed r."""
        c0, c1 = self.cs(s0, s1)
        q0 = e0 * 2 + r
        fl = self.fl()
        return fl[:, q0:q0 + 2 * estep * (n - 1) + 1:2 * estep, c0:c1]

    def all_ap(self, s0, s1):
        c0, c1 = self.cs(s0, s1)
        return self.fl()[:, :, c0:c1]


def build_program():
    import concourse.bacc as bacc
    import concourse.mybir as mybir
    from concourse.tile import TileContext
    FP = mybir.dt.float32
    AL = mybir.AluOpType

    nc = bacc.Bacc("TRN2", target_bir_lowering=False, debug=False)
    fld_d = nc.declare_dram_parameter("fld", [128, 24 * NSH * SL], FP, isOutput=False)
    gg_d = nc.declare_dram_parameter("gg", [128, 72 * NSH * SL], FP, isOutput=False)
    out_d = nc.declare_dram_parameter("outp", [128, 24 * TLOC * SL], FP, isOutput=True)

    with TileContext(nc) as tc:
        _tc = [0]

        def mk(pool, cols, tag):
            _tc[0] += 1
            return pool.tile([128, cols], FP, tag=tag, name=f"{tag}_{_tc[0]}")

        main_cm = tc.tile_pool(name="main", bufs=1)
        main = main_cm.__enter__()
        gpool_cm = tc.tile_pool(name="gp", bufs=1)
        gpool = gpool_cm.__enter__()
        spool_cm = tc.tile_pool(name="sp", bufs=1)
        spool = spool_cm.__enter__()

        F = Grp(mk(main, 24 * NSH * SL, "F"), 12, 0, NSH)
        O = Grp(mk(main, 24 * TLOC * SL, "O"), 12, 1, TLOC)
        nc.sync.dma_start(out=F.t[:, :], in_=fld_d[:, :])

        def load_dir(k, tag):
            g = Grp(mk(gpool, 9 * 2 * NSH * SL, tag), 9, 0, NSH)
            nc.sync.dma_start(out=g.t[:, :],
                              in_=gg_d[:, k * 18 * NSH * SL:(k + 1) * 18 * NSH * SL])
            return g

        # scratch tiles (max range 5 slices = 160 cols), per compute engine
        WMX = 5 * SL
        t1 = mk(spool, 9 * WMX, "mt1")
        t2 = mk(spool, 9 * WMX, "mt2")
        su = mk(spool, 3 * WMX, "ms")
        su2 = mk(spool, 3 * WMX, "mu")
        t1g = mk(spool, 9 * WMX, "mt1g")
        t2g = mk(spool, 9 * WMX, "mt2g")
        sug = mk(spool, 3 * WMX, "msg")
        su2g = mk(spool, 3 * WMX, "mug")

        def tview(t, nj, nk, w):
            return t[:, :nj * nk * w].rearrange("p (j k w) -> p j k w",
                                                j=nj, k=nk, w=w)

        def sview(t, nj, w):
            return t[:, :nj * w].rearrange("p (j w) -> p j w", j=nj, w=w)

        V = nc.vector

        def a_pack(A, i, adag, r, s0, s1, dt, nj):
            """[p, nj(bcast), 3, w] for a-values (i,k)."""
            e0, st = (i, 3) if adag else (i * 3, 1)
            ap = A.gap(e0, st, 3, r, s0 + dt, s1 + dt)       # [p,3,w]
            w = ap.shape[2]
            return ap.unsqueeze(1).broadcast_to((128, nj, 3, w))

        def b_pack(B, bdag, r, s0, s1, dt, nj):
            """[p, nj, 3, w] for b-values (k,j)."""
            c0, c1 = B.cs(s0 + dt, s1 + dt)
            w = c1 - c0
            if bdag:  # e = j*3+k
                v = B.t.rearrange("p (j k r w) -> p j k r w", j=3, k=3, r=2, w=B.W)
                return v[:, :, :, r, c0:c1]
            if B.nent == 9:  # e = k*3+j
                v = B.t.rearrange("p (k j r w) -> p k j r w", k=3, j=3, r=2, w=B.W)
                return v[:, :, :, r, c0:c1].transpose([0, 2, 1, 3])
            # halfspinor: e = k*2+j, nj=2
            v = B.t.rearrange("p (k j r w) -> p k j r w", k=3, j=2, r=2, w=B.W)
            return v[:, :, :, r, c0:c1].transpose([0, 2, 1, 3])

        def stt(out, in0, coef, in1, op1=None, eng=None):
            (eng or V).scalar_tensor_tensor(out, in0, float(coef), in1,
                                            AL.mult, op1 or AL.add)

        # engine + scratch per output row i of a product (GPSIMD takes i=2)
        ENG_ROWS = [(V, t1, t2, su, su2), (V, t1, t2, su, su2),
                    (nc.gpsimd, t1g, t2g, sug, su2g) if ENGINE_MIX
                    else (V, t1, t2, su, su2)]

        def cmm(dst, A, B, s0, s1, adag=False, bdag=False, adt=0, bdt=0, nj=3):
            """dst[i,j] = sum_k aval(i,k)*bval(k,j); dst entries e=i*nj+j.
            No (adag and bdag) case: signs reduce to add/sub combines."""
            assert not (adag and bdag)
            w = (s1 - s0) * SL
            for i in range(3):
                E, x1, x2, xs, xu = ENG_ROWS[i]
                tv1, tv2 = tview(x1, nj, 3, w), tview(x2, nj, 3, w)
                sv, uv = sview(xs, nj, w), sview(xu, nj, w)
                ar = a_pack(A, i, adag, 0, s0, s1, adt, nj)
                ai = a_pack(A, i, adag, 1, s0, s1, adt, nj)
                br = b_pack(B, bdag, 0, s0, s1, bdt, nj)
                bi = b_pack(B, bdag, 1, s0, s1, bdt, nj)
                dre = dst.gap(i * nj, 1, nj, 0, s0, s1)
                dim = dst.gap(i * nj, 1, nj, 1, s0, s1)

                def red(tv, out):
                    E.tensor_add(out, tv[:, :, 0, :], tv[:, :, 1, :])
                    E.tensor_add(out, out, tv[:, :, 2, :])
                # real: Srr - sa*sb*Sii
                E.tensor_mul(tv1, ar, br)
                E.tensor_mul(tv2, ai, bi)
                red(tv1, sv)
                red(tv2, uv)
                if adag or bdag:
                    E.tensor_add(dre, sv, uv)
                else:
                    E.tensor_sub(dre, sv, uv)
                # imag: sb*Sri + sa*Sir
                E.tensor_mul(tv1, ar, bi)
                E.tensor_mul(tv2, ai, br)
                red(tv1, sv)
                red(tv2, uv)
                if adag:
                    E.tensor_sub(dim, sv, uv)
                elif bdag:
                    E.tensor_sub(dim, uv, sv)
                else:
                    E.tensor_add(dim, sv, uv)

        # ---------- shift materialization ----------
        def xview(g, s0, s1):
            c0, c1 = g.cs(s0, s1)
            nb = (c1 - c0) // 16
            v = g.t.rearrange("p (q b x) -> p q b x", q=g.nent * 2,
                              b=g.ns * 2, x=16)
            return v[:, :, (c0 // 16):(c0 // 16) + nb, :]

        def cpy(out, in_):
            if ENGINE_MIX:
                nc.scalar.copy(out, in_)
            else:
                V.tensor_copy(out, in_)

        def mat_shift(src, axis, d, s0, s1, tag, pool=None):
            """materialize S(x)=src(x + d*e_axis) over the FULL src range
            (so DMA src/dst APs share structure). axis 1,2,3."""
            g = Grp(mk(pool or main, src.nent * 2 * src.ns * SL, tag),
                    src.nent, src.base, src.ns)
            sb, se = src.base, src.base + src.ns
            if axis == 3:   # X
                def xv(t_):
                    return t_.rearrange("p (m x) -> p m x", x=16)
                dv, sv_ = xv(g.t), xv(src.t)
                if d == +1:
                    cpy(dv[:, :, 0:15], sv_[:, :, 1:16])
                    cpy(dv[:, :, 15:16], sv_[:, :, 0:1])
                else:
                    cpy(dv[:, :, 1:16], sv_[:, :, 0:15])
                    cpy(dv[:, :, 0:1], sv_[:, :, 15:16])
            elif axis == 1:  # Z: partition +-8
                sall, dall = src.t[:, :], g.t[:, :]
                if d == +1:
                    nc.sync.dma_start(out=dall[0:120], in_=sall[8:128])
                    nc.sync.dma_start(out=dall[120:128], in_=sall[0:8])
                else:
                    nc.sync.dma_start(out=dall[8:128], in_=sall[0:120])
                    nc.sync.dma_start(out=dall[0:8], in_=sall[120:128])
            else:           # Y
                def lv(t_, lo):
                    return t_.rearrange("p (m x) -> p m x", x=16)[:, lo::2, :]
                if d == +1:
                    # lo=0 out <- lo=1 in (same p)
                    cpy(lv(g.t, 0), lv(src.t, 1))
                    # lo=1 out <- lo=0 in at p+1; wrap h=7 <- h=0 same z
                    nc.sync.dma_start(out=lv(g.t[0:127, :], 1),
                                      in_=lv(src.t[1:128, :], 0))
                    for z in range(16):
                        nc.sync.dma_start(
                            out=lv(g.t[z * 8 + 7:z * 8 + 8, :], 1),
                            in_=lv(src.t[z * 8:z * 8 + 1, :], 0))
                else:
                    cpy(lv(g.t, 1), lv(src.t, 0))
                    nc.sync.dma_start(out=lv(g.t[1:128, :], 0),
                                      in_=lv(src.t[0:127, :], 1))
                    for z in range(16):
                        nc.sync.dma_start(
                            out=lv(g.t[z * 8:z * 8 + 1, :], 0),
                            in_=lv(src.t[z * 8 + 7:z * 8 + 8, :], 1))
            return g

        # ---------- mass term: O = (MASS+4) * F ----------
        V.tensor_scalar_mul(O.all_ap(1, 5), F.all_ap(1, 5), float(MASS + 4.0))

        # ---------- Wilson hops ----------
        with tc.tile_pool(name="wp", bufs=1) as wpool:
            for k in range(4):
                ax = k  # lattice axis
                Uk = load_dir(k, "glA")
                # ---- forward hop ----
                rng = (0, 4) if k == 0 else (1, 5)
                s0, s1 = rng
                h = Grp(mk(wpool, 12 * (s1 - s0) * SL, "h"),
                        6, s0, s1 - s0)
                # h[c,j] = psi[c,j] + g(j)*psi[c,p(j)]
                for j in range(2):
                    pj, gj = GCOL[k][j]
                    for r in range(2):
                        pr, psgn = (_phase_parts(gj)[0:2] if r == 0
                                    else _phase_parts(gj)[2:4])
                        dst = h.gap(j, 2, 3, r, s0, s1)       # c-packed
                        a0 = F.gap(j, 4, 3, r, s0, s1)        # psi[c,j].r
                        a1 = F.gap(pj, 4, 3, pr, s0, s1)
                        stt(dst, a1, psgn, a0, AL.add)
                phi = Grp(mk(wpool, 12 * (s1 - s0) * SL, "phi"),
                          6, s0, s1 - s0)
                cmm(phi, Uk, h, s0, s1, adag=True, nj=2)
                # shift (-1 along ax) then reconstruct into O
                if k == 0:
                    psh, dt = phi, -1
                else:
                    psh, dt = mat_shift(phi, ax, -1, 1, 5, "psh", wpool), 0
                # out[:, s'] += -0.5 * rho(s') * psh[:, q(s')]
                rec = [(0, 1.0), (1, 1.0), PROJ_F[k][0], PROJ_F[k][1]]
                for sp in range(4):
                    q, rho = rec[sp]
                    for r in range(2):
                        pr, psgn = (_phase_parts(rho)[0:2] if r == 0
                                    else _phase_parts(rho)[2:4])
                        dst = O.gap(sp, 4, 3, r, 1, 5)
                        src_ = psh.gap(q, 2, 3, pr, 1 + dt, 5 + dt)
                        stt(dst, src_, -0.5 * psgn, dst, AL.add)
                # ---- backward hop ----
                rng = (2, 6) if k == 0 else (1, 5)
                s0, s1 = rng
                hb = Grp(mk(wpool, 12 * (s1 - s0) * SL, "h"),
                         6, s0, s1 - s0)
                # hb[c,j] = g(j)*psi[c,p(j)] - psi[c,j]
                for j in range(2):
                    pj, gj = GCOL[k][j]
                    for r in range(2):
                        pr, psgn = (_phase_parts(gj)[0:2] if r == 0
                                    else _phase_parts(gj)[2:4])
                        dst = hb.gap(j, 2, 3, r, s0, s1)
                        a1 = F.gap(pj, 4, 3, pr, s0, s1)
                        a0 = F.gap(j, 4, 3, r, s0, s1)
                        # dst = psgn*a1 - a0  -> stt: (a0 * -1) + ...? need scaled a1.
                        # use: dst = (a1*psgn) + (-a0): two-step via subtract:
                        # dst = (a1 * psgn) sub? op1 options: use subtract_rev?
                        # simplest: dst = (a1*psgn) + a0*(-1): do STT then sub.
                        stt(dst, a1, psgn, a0, AL.subtract)
                if k == 0:
                    hs, dt = hb, +1
                else:
                    hs, dt = mat_shift(hb, ax, +1, 1, 5, "psh", wpool), 0
                gm = Grp(mk(wpool, 12 * 4 * SL, "phi"),
                         6, 1, 4)
                cmm(gm, Uk, hs, 1, 5, nj=2, bdt=dt)
                rec = [(0, 1.0), (1, 1.0), PROJ_B[k][0], PROJ_B[k][1]]
                for sp in range(4):
                    q, rho = rec[sp]
                    for r in range(2):
                        pr, psgn = (_phase_parts(rho)[0:2] if r == 0
                                    else _phase_parts(rho)[2:4])
                        dst = O.gap(sp, 4, 3, r, 1, 5)
                        src_ = gm.gap(q, 2, 3, pr, 1, 5)
                        stt(dst, src_, 0.5 * psgn, dst, AL.add)

        # ---------- clover planes ----------
        with tc.tile_pool(name="cp", bufs=1) as cpool:
            def ctile(tag, nent, s0, s1):
                return Grp(mk(cpool, nent * 2 * (s1 - s0) * SL, tag), nent, s0, s1 - s0)

            for d1 in range(1, 5):
                for d2 in range(d1 + 1, 5):
                    a1, a2 = d1 - 1, d2 - 1
                    tpl = (d1 == 1)
                    sA, eA = (0, 5) if tpl else (1, 5)
                    U1 = load_dir(d1 - 1, "glA")
                    U2 = load_dir(d2 - 1, "glB")
                    # shifted links
                    if tpl:
                        U2m, u2dt = U2, +1       # U_d2(x+e_T): column view
                    else:
                        U2m, u2dt = mat_shift(U2, a1, +1, sA, eA, "lnk1", cpool), 0
                    U1n = mat_shift(U1, a2, +1, sA, eA, "lnk2", cpool)
                    A = ctile("pA", 9, sA, eA)
                    cmm(A, U1, U2m, sA, eA, bdt=u2dt)
                    B = ctile("pB", 9, sA, eA)
                    cmm(B, U2, U1n, sA, eA)
                    Q = ctile("pQ", 9, 1, 5)
                    cmm(Q, B, A, 1, 5, bdag=True)            # L4
                    sL1, eL1 = (0, 4) if tpl else (1, 5)
                    L1 = ctile("pL", 9, sL1, eL1)
                    cmm(L1, A, B, sL1, eL1, adag=True)
                    # Q += shift_{-a1,-a2}(L1)
                    if tpl:
                        L1s = mat_shift(L1, a2, -1, 0, 4, "pLs", cpool)
                        V.tensor_add(Q.all_ap(1, 5), Q.all_ap(1, 5),
                                     L1s.all_ap(0, 4))
                    else:
                        L1s = mat_shift(L1, a1, -1, 1, 5, "pLs", cpool)
                        L1ss = mat_shift(L1s, a2, -1, 1, 5, "pLss", cpool)
                        V.tensor_add(Q.all_ap(1, 5), Q.all_ap(1, 5),
                                     L1ss.all_ap(1, 5))
                    D = ctile("pA", 9, sA, eA)
                    cmm(D, U1n, U2m, sA, eA, bdag=True, bdt=u2dt)
                    E = ctile("pB", 9, sA, eA)
                    cmm(E, U2, U1, sA, eA, adag=True)
                    L2 = ctile("pL", 9, 1, 5)
                    cmm(L2, D, E, 1, 5, bdag=True)
                    L2s = mat_shift(L2, a2, -1, 1, 5, "pLs", cpool)
                    V.tensor_add(Q.all_ap(1, 5), Q.all_ap(1, 5),
                                 L2s.all_ap(1, 5))
                    sL3, eL3 = (0, 4) if tpl else (1, 5)
                    L3 = ctile("pL", 9, sL3, eL3)
                    cmm(L3, E, D, sL3, eL3, adag=True)
                    if tpl:
                        V.tensor_add(Q.all_ap(1, 5), Q.all_ap(1, 5),
                                     L3.all_ap(0, 4))
                    else:
                        L3s = mat_shift(L3, a1, -1, 1, 5, "pLs", cpool)
                        V.tensor_add(Q.all_ap(1, 5), Q.all_ap(1, 5),
                                     L3s.all_ap(1, 5))
                    # ---- G9 = Q - Q^dag ----
                    G9 = ctile("pG9", 9, 1, 5)
                    qv = Q.t.rearrange("p (i j r w) -> p i j r w",
                                       i=3, j=3, r=2, w=Q.W)
                    qT = qv.transpose([0, 2, 1, 3, 4])
                    gv = G9.t.rearrange("p (i j r w) -> p i j r w",
                                        i=3, j=3, r=2, w=G9.W)
                    V.tensor_sub(gv[:, :, :, 0, :], qv[:, :, :, 0, :],
                                 qT[:, :, :, 0, :])
                    V.tensor_add(gv[:, :, :, 1, :], qv[:, :, :, 1, :],
                                 qT[:, :, :, 1, :])
                    # ---- apply: O -= 1/16 * G9 * (psi sigma) ----
                    w4 = 4 * SL
                    tg1 = tview(t1, 3, 3, w4)
                    tg2 = tview(t2, 3, 3, w4)
                    sv4, uv4 = sview(su, 3, w4), sview(su2, 3, w4)
                    for sp in range(4):
                        wrow, phi_ph = SIGCOL[a1][a2][sp]
                        vr_p, vr_s, vi_p, vi_s = _phase_parts(phi_ph)
                        # v.re plane = vr_s * psi[j, wrow].(vr_p)
                        for outr in range(2):
                            # out.re += sum_j gr*v.re - gi*v.im
                            # out.im += sum_j gr*v.im + gi*v.re
                            if outr == 0:
                                pa, sa_ = vr_p, vr_s
                                pb, sb_ = vi_p, -vi_s
                            else:
                                pa, sa_ = vi_p, vi_s
                                pb, sb_ = vr_p, vr_s
                            # psi plane for color j: entry j*4+wrow, part pa;
                            # broadcast over i (t layout [p, i, j, w])
                            psA = F.gap(wrow, 4, 3, pa, 1, 5)   # [p,3(j),w]
                            psB = F.gap(wrow, 4, 3, pb, 1, 5)
                            psA4 = psA.unsqueeze(1).broadcast_to((128, 3, 3, w4))
                            psB4 = psB.unsqueeze(1).broadcast_to((128, 3, 3, w4))
                            V.tensor_mul(tg1, gv[:, :, :, 0, :], psA4)
                            V.tensor_mul(tg2, gv[:, :, :, 1, :], psB4)
                            # d = sa_*t1 + sb_*t2 ; reduce over j ; O += -1/16*d
                            if sa_ * sb_ > 0:
                                V.tensor_add(tg1, tg1, tg2)
                            else:
                                V.tensor_sub(tg1, tg1, tg2)
                            V.tensor_add(sv4, tg1[:, :, 0, :], tg1[:, :, 1, :])
                            V.tensor_add(sv4, sv4, tg1[:, :, 2, :])
                            dst = O.gap(sp, 4, 3, outr, 1, 5)
                            stt(dst, sv4, -(1.0 / 16.0) * sa_, dst, AL.add)

        nc.sync.dma_start(out=out_d[:, :], in_=O.t[:, :])
        spool_cm.__exit__(None, None, None)
        gpool_cm.__exit__(None, None, None)
        main_cm.__exit__(None, None, None)

    nc.compile()
    return nc


# ----------------------------------------------------------------------
# host entry
# ----------------------------------------------------------------------
_CACHE = {}

def _get_nc():
    if "nc" not in _CACHE:
        _CACHE["nc"] = build_program()
    return _CACHE["nc"]


def kernel(field_re, field_im, gauge_re, gauge_im):
    from concourse.bass_utils import run_bass_kernel_spmd
    nc = _get_nc()
    in_maps = [_host_inputs(field_re, field_im, gauge_re, gauge_im, c)
               for c in range(NCORES)]
    br = run_bass_kernel_spmd(nc, in_maps, list(range(NCORES)))
    out = np.empty((T_, Z_, Y_, X_, 3, 4, 2), np.float32)
    for c in range(NCORES):
        out[c * TLOC:(c + 1) * TLOC] = _host_output(br.results[c]["outp"])
    return out


# revision 27
# speedup vs baseline: 1.3398x; 1.3398x over previous
"""Wilson-clover Dirac operator D_WC on Trainium2, 8-core SPMD.

Self-contained: hardcodes LAT=(32,16,16,16), shards the T axis across 8
cores with host-side halo slices (t0-1..t0+4), computes everything
site-locally on-device with DVE elementwise math in an SoA layout:

  plane[p, col]: p = z*8 + (y>>1),  col = plane_idx*W + (s-base)*32
                 + (y&1)*16 + x          (s = local T-slice, W = ns*32)

Shifts: T = column offset (free), X = 2-piece AP copy, Z = 2-piece
partition-offset DMA, Y = 1 copy + 2 DMAs.
"""
import numpy as np

# ----------------------------------------------------------------------
# constants (physics)
# ----------------------------------------------------------------------
MASS, C_SW = 0.5, 1.0
_i = 1j
G1 = np.array([[0,0,0,_i],[0,0,_i,0],[0,-_i,0,0],[-_i,0,0,0]], dtype=np.complex64)
G2 = np.array([[0,0,0,-1],[0,0,1,0],[0,1,0,0],[-1,0,0,0]], dtype=np.complex64)
G3 = np.array([[0,0,_i,0],[0,0,0,-_i],[-_i,0,0,0],[0,_i,0,0]], dtype=np.complex64)
G4 = np.array([[0,0,1,0],[0,0,0,1],[1,0,0,0],[0,1,0,0]], dtype=np.complex64)
GAMMA = [G1, G2, G3, G4]
I4 = np.eye(4, dtype=np.complex64)
SIGMA = [[(0.5j * (GAMMA[m] @ GAMMA[n] - GAMMA[n] @ GAMMA[m])).astype(np.complex64)
          for n in range(4)] for m in range(4)]

T_, Z_, Y_, X_ = 32, 16, 16, 16
NCORES, TLOC, NSH = 8, 4, 6       # halo slices per core
SL = 32                           # cols per slice
ENGINE_MIX = True                 # offload product rows to GPSIMD, copies to ACT
USE_BF16 = True                   # bf16 compute tiles (O stays fp32)
SIM_SAFE = False                  # True: per-z Y-wrap DMAs (CoreSim-compatible)
F32 = None                        # set after mybir import (device build only)

# ----------------------------------------------------------------------
# spin-structure extraction (numerical)
# ----------------------------------------------------------------------
def _col_struct(M):
    """M has single-nonzero columns: return per-col (row, phase)."""
    out = []
    for s in range(4):
        col = M[:, s]
        r = int(np.argmax(np.abs(col)))
        ph = complex(col[r])
        assert np.sum(np.abs(col) > 1e-6) == 1, (M, s)
        out.append((r, ph))
    return out

def _proj_struct(P):
    """P rank-2 with cols 2,3 = rho * cols q in {0,1}. Returns (q2,rho2,q3,rho3)."""
    res = []
    for s in (2, 3):
        found = None
        for j in (0, 1):
            c, cj = P[:, s], P[:, j]
            nz = np.abs(cj) > 1e-6
            if not nz.any():
                continue
            ratio = c[nz] / cj[nz]
            if np.allclose(ratio, ratio[0], atol=1e-5) and np.allclose(
                    c, ratio[0] * cj, atol=1e-5):
                found = (j, complex(ratio[0]))
                break
        assert found is not None, P
        res.append(found)
    return res

# per direction k: gamma column structure and projector relations
GCOL = [_col_struct(GAMMA[k]) for k in range(4)]           # (p(s), g(s))
PROJ_F = [_proj_struct(GAMMA[k] + I4) for k in range(4)]   # for psi@(G+I)
PROJ_B = [_proj_struct(GAMMA[k] - I4) for k in range(4)]   # for psi@(G-I)
SIGCOL = [[_col_struct(SIGMA[m][n]) if m != n else None for n in range(4)]
          for m in range(4)]

def _phase_parts(ph):
    """phase in {1,-1,i,-i} -> for v = ph*u:
    re(v) = sr*u.[pr]  im(v) = si*u.[pi]   (0=re,1=im planes of u)"""
    if abs(ph - 1) < 1e-5:   return (0, 1.0, 1, 1.0)
    if abs(ph + 1) < 1e-5:   return (0, -1.0, 1, -1.0)
    if abs(ph - 1j) < 1e-5:  return (1, -1.0, 0, 1.0)
    if abs(ph + 1j) < 1e-5:  return (1, 1.0, 0, -1.0)
    raise AssertionError(ph)

# ----------------------------------------------------------------------
# host layout helpers
# ----------------------------------------------------------------------
def _to_planes(vol):
    """vol [ns, Z, Y, X] -> [128, ns*32]"""
    ns = vol.shape[0]
    v = vol.reshape(ns, 16, 8, 2, 16)        # s z yh yl x
    v = np.transpose(v, (1, 2, 0, 3, 4))     # z yh s yl x
    return np.ascontiguousarray(v.reshape(128, ns * 32))

def _from_planes(pl, ns):
    v = pl.reshape(16, 8, ns, 2, 16)
    return np.transpose(v, (2, 0, 1, 3, 4)).reshape(ns, 16, 16, 16)

def _host_inputs(field_re, field_im, gauge_re, gauge_im, core):
    t0 = core * TLOC
    ts = [(t0 - 1 + s) % T_ for s in range(NSH)]
    f = np.stack([field_re[ts], field_im[ts]], axis=0)   # [2,6,Z,Y,X,3,4]
    g = np.stack([gauge_re[:, ts], gauge_im[:, ts]], axis=0)  # [2,4,6,Z,Y,X,3,3]

    # field planes: e=(c*4+sp), plane=e*2+r
    fp = np.empty((24, 128, NSH * SL), np.float32)
    for c in range(3):
        for sp in range(4):
            for r in range(2):
                fp[(c * 4 + sp) * 2 + r] = _to_planes(f[r, :, :, :, :, c, sp])
    fld = np.ascontiguousarray(fp.transpose(1, 0, 2).reshape(128, 24 * NSH * SL))

    # gauge planes: dir k, e=(a*3+b), plane=(k*9+e)*2+r
    gp = np.empty((72, 128, NSH * SL), np.float32)
    for k in range(4):
        for a in range(3):
            for b in range(3):
                for r in range(2):
                    gp[(k * 9 + a * 3 + b) * 2 + r] = _to_planes(
                        g[r, k, :, :, :, :, a, b])
    gg = np.ascontiguousarray(gp.transpose(1, 0, 2).reshape(128, 72 * NSH * SL))
    if USE_BF16:
        import ml_dtypes
        fld = fld.astype(ml_dtypes.bfloat16)
        gg = gg.astype(ml_dtypes.bfloat16)
    return {"fld": fld, "gg": gg}

def _host_output(outp_flat):
    """device out [128, 24*4*32] -> [TLOC, Z,Y,X, 3,4,2]"""
    pl = outp_flat.reshape(128, 24, TLOC * SL).transpose(1, 0, 2)
    out = np.empty((TLOC, 16, 16, 16, 3, 4, 2), np.float32)
    for c in range(3):
        for sp in range(4):
            for r in range(2):
                out[..., c, sp, r] = _from_planes(pl[(c * 4 + sp) * 2 + r], TLOC)
    return out

# ----------------------------------------------------------------------
# device program
# ----------------------------------------------------------------------
class Grp:
    """Group of planes in one SBUF tile. nent complex entries (re+im planes).
    base = slice index of col 0; ns slices; W = ns*32 cols per plane."""
    def __init__(self, tile, nent, base, ns):
        self.t, self.nent, self.base, self.ns = tile, nent, base, ns
        self.W = ns * SL

    def fl(self):
        return self.t.rearrange("p (q w) -> p q w", q=self.nent * 2, w=self.W)

    def cs(self, s0, s1):
        return ((s0 - self.base) * SL, (s1 - self.base) * SL)

    def pap(self, e, r, s0, s1):
        """single plane AP [128, cols]"""
        c0, c1 = self.cs(s0, s1)
        v = self.fl()[:, (e * 2 + r):(e * 2 + r + 1), c0:c1]
        return v  # [p,1,w]

    def gap(self, e0, estep, n, r, s0, s1):
        """packed-entry AP [p, n, w]: entries e0 + i*estep, fixed r."""
        c0, c1 = self.cs(s0, s1)
        q0 = e0 * 2 + r
        fl = self.fl()
        return fl[:, q0:q0 + 2 * estep * (n - 1) + 1:2 * estep, c0:c1]

    def all_ap(self, s0, s1):
        c0, c1 = self.cs(s0, s1)
        return self.fl()[:, :, c0:c1]


def build_program():
    import concourse.bacc as bacc
    import concourse.mybir as mybir
    from concourse.tile import TileContext
    FP = mybir.dt.float32
    CDT = mybir.dt.bfloat16 if USE_BF16 else FP
    AL = mybir.AluOpType

    nc = bacc.Bacc("TRN2", target_bir_lowering=False, debug=False)
    fld_d = nc.declare_dram_parameter("fld", [128, 24 * NSH * SL], CDT, isOutput=False)
    gg_d = nc.declare_dram_parameter("gg", [128, 72 * NSH * SL], CDT, isOutput=False)
    out_d = nc.declare_dram_parameter("outp", [128, 24 * TLOC * SL], FP, isOutput=True)

    with TileContext(nc) as tc:
        _tc = [0]

        def mk(pool, cols, tag, dt=None):
            _tc[0] += 1
            return pool.tile([128, cols], dt or CDT, tag=tag,
                             name=f"{tag}_{_tc[0]}")

        main_cm = tc.tile_pool(name="main", bufs=1)
        main = main_cm.__enter__()
        gpool_cm = tc.tile_pool(name="gp", bufs=1)
        gpool = gpool_cm.__enter__()
        spool_cm = tc.tile_pool(name="sp", bufs=1)
        spool = spool_cm.__enter__()

        F = Grp(mk(main, 24 * NSH * SL, "F"), 12, 0, NSH)
        O = Grp(mk(main, 24 * TLOC * SL, "O", FP), 12, 1, TLOC)
        nc.sync.dma_start(out=F.t[:, :], in_=fld_d[:, :])

        def load_dir(k, tag):
            g = Grp(mk(gpool, 9 * 2 * NSH * SL, tag), 9, 0, NSH)
            nc.sync.dma_start(out=g.t[:, :],
                              in_=gg_d[:, k * 18 * NSH * SL:(k + 1) * 18 * NSH * SL])
            return g

        # merged scratch tiles (2 halves each), per compute engine
        WMX = 5 * SL
        mtA = mk(spool, 2 * 9 * WMX, "mtA")
        msA = mk(spool, 2 * 3 * WMX, "msA")
        mtG = mk(spool, 2 * 9 * WMX, "mtG")
        msG = mk(spool, 2 * 3 * WMX, "msG")

        def tview2(t, nj, nk, w):
            """[p, 2, nj, nk, w] halves of merged mul scratch"""
            v = t.rearrange("p (h m) -> p h m", h=2)
            return v[:, :, :nj * nk * w].rearrange(
                "p h (j k w) -> p h j k w", j=nj, k=nk, w=w)

        def sview2(t, nj, w):
            v = t.rearrange("p (h m) -> p h m", h=2)
            return v[:, :, :nj * w].rearrange("p h (j w) -> p h j w",
                                              j=nj, w=w)

        V = nc.vector

        def a_pack(A, i, adag, r, s0, s1, dt, nj):
            """[p, nj(bcast), 3, w] for a-values (i,k)."""
            e0, st = (i, 3) if adag else (i * 3, 1)
            ap = A.gap(e0, st, 3, r, s0 + dt, s1 + dt)       # [p,3,w]
            w = ap.shape[2]
            return ap.unsqueeze(1).broadcast_to((128, nj, 3, w))

        def b_pack(B, bdag, r, s0, s1, dt, nj):
            """[p, nj, 3, w] for b-values (k,j)."""
            c0, c1 = B.cs(s0 + dt, s1 + dt)
            w = c1 - c0
            if bdag:  # e = j*3+k
                v = B.t.rearrange("p (j k r w) -> p j k r w", j=3, k=3, r=2, w=B.W)
                return v[:, :, :, r, c0:c1]
            if B.nent == 9:  # e = k*3+j
                v = B.t.rearrange("p (k j r w) -> p k j r w", k=3, j=3, r=2, w=B.W)
                return v[:, :, :, r, c0:c1].transpose([0, 2, 1, 3])
            # halfspinor: e = k*2+j, nj=2
            v = B.t.rearrange("p (k j r w) -> p k j r w", k=3, j=2, r=2, w=B.W)
            return v[:, :, :, r, c0:c1].transpose([0, 2, 1, 3])

        def stt(out, in0, coef, in1, op1=None, eng=None):
            (eng or V).scalar_tensor_tensor(out, in0, float(coef), in1,
                                            AL.mult, op1 or AL.add)

        DV_SET = (V, mtA, msA)
        GP_SET = (nc.gpsimd, mtG, msG)
        _pc = [0]  # product counter for gp alternation

        def cmm(dst, A, B, s0, s1, adag=False, bdag=False, adt=0, bdt=0, nj=3):
            """dst[i,j] = sum_k aval(i,k)*bval(k,j); dst entries e=i*nj+j.
            No (adag and bdag) case: signs reduce to add/sub combines.
            Row i=2 goes to GPSIMD for alternating products (ENGINE_MIX)."""
            assert not (adag and bdag)
            _pc[0] += 1
            use_gp = ENGINE_MIX and (_pc[0] % 2 == 0)
            w = (s1 - s0) * SL
            for i in range(3):
                E, xt, xs = GP_SET if (use_gp and i == 2) else DV_SET
                tv = tview2(xt, nj, 3, w)      # [p,2,nj,3,w]
                sv = sview2(xs, nj, w)         # [p,2,nj,w]
                ar = a_pack(A, i, adag, 0, s0, s1, adt, nj)
                ai = a_pack(A, i, adag, 1, s0, s1, adt, nj)
                br = b_pack(B, bdag, 0, s0, s1, bdt, nj)
                bi = b_pack(B, bdag, 1, s0, s1, bdt, nj)
                dre = dst.gap(i * nj, 1, nj, 0, s0, s1)
                dim = dst.gap(i * nj, 1, nj, 1, s0, s1)
                # real: Srr - sa*sb*Sii
                E.tensor_mul(tv[:, 0], ar, br)
                E.tensor_mul(tv[:, 1], ai, bi)
                E.tensor_add(sv, tv[:, :, :, 0, :], tv[:, :, :, 1, :])
                E.tensor_add(sv, sv, tv[:, :, :, 2, :])
                if adag or bdag:
                    E.tensor_add(dre, sv[:, 0], sv[:, 1])
                else:
                    E.tensor_sub(dre, sv[:, 0], sv[:, 1])
                # imag: sb*Sri + sa*Sir
                E.tensor_mul(tv[:, 0], ar, bi)
                E.tensor_mul(tv[:, 1], ai, br)
                E.tensor_add(sv, tv[:, :, :, 0, :], tv[:, :, :, 1, :])
                E.tensor_add(sv, sv, tv[:, :, :, 2, :])
                if adag:
                    E.tensor_sub(dim, sv[:, 0], sv[:, 1])
                elif bdag:
                    E.tensor_sub(dim, sv[:, 1], sv[:, 0])
                else:
                    E.tensor_add(dim, sv[:, 0], sv[:, 1])

        # ---------- shift materialization ----------
        def xview(g, s0, s1):
            c0, c1 = g.cs(s0, s1)
            nb = (c1 - c0) // 16
            v = g.t.rearrange("p (q b x) -> p q b x", q=g.nent * 2,
                              b=g.ns * 2, x=16)
            return v[:, :, (c0 // 16):(c0 // 16) + nb, :]

        def cpy(out, in_):
            if ENGINE_MIX:
                nc.scalar.copy(out, in_)
            else:
                V.tensor_copy(out, in_)

        def mat_shift(src, axis, d, s0, s1, tag, pool=None):
            """materialize S(x)=src(x + d*e_axis) over the FULL src range
            (so DMA src/dst APs share structure). axis 1,2,3."""
            g = Grp(mk(pool or main, src.nent * 2 * src.ns * SL, tag),
                    src.nent, src.base, src.ns)
            sb, se = src.base, src.base + src.ns
            if axis == 3:   # X
                def xv(t_):
                    return t_.rearrange("p (m x) -> p m x", x=16)
                dv, sv_ = xv(g.t), xv(src.t)
                if d == +1:
                    cpy(dv[:, :, 0:15], sv_[:, :, 1:16])
                    cpy(dv[:, :, 15:16], sv_[:, :, 0:1])
                else:
                    cpy(dv[:, :, 1:16], sv_[:, :, 0:15])
                    cpy(dv[:, :, 0:1], sv_[:, :, 15:16])
            elif axis == 1:  # Z: partition +-8
                sall, dall = src.t[:, :], g.t[:, :]
                if d == +1:
                    nc.sync.dma_start(out=dall[0:120], in_=sall[8:128])
                    nc.sync.dma_start(out=dall[120:128], in_=sall[0:8])
                else:
                    nc.sync.dma_start(out=dall[8:128], in_=sall[0:120])
                    nc.sync.dma_start(out=dall[0:8], in_=sall[120:128])
            else:           # Y
                def lv(t_, lo):
                    return t_.rearrange("p (m x) -> p m x", x=16)[:, lo::2, :]
                if d == +1:
                    # lo=0 out <- lo=1 in (same p)
                    cpy(lv(g.t, 0), lv(src.t, 1))
                    # lo=1 out <- lo=0 in at p+1; wrap h=7 <- h=0 same z
                    nc.sync.dma_start(out=lv(g.t[0:127, :], 1),
                                      in_=lv(src.t[1:128, :], 0))
                    if SIM_SAFE:
                        for z in range(16):
                            nc.sync.dma_start(
                                out=lv(g.t[z * 8 + 7:z * 8 + 8, :], 1),
                                in_=lv(src.t[z * 8:z * 8 + 1, :], 0))
                    else:
                        nc.sync.dma_start(out=lv(g.t[7:128:8, :], 1),
                                          in_=lv(src.t[0:128:8, :], 0))
                else:
                    cpy(lv(g.t, 1), lv(src.t, 0))
                    nc.sync.dma_start(out=lv(g.t[1:128, :], 0),
                                      in_=lv(src.t[0:127, :], 1))
                    if SIM_SAFE:
                        for z in range(16):
                            nc.sync.dma_start(
                                out=lv(g.t[z * 8:z * 8 + 1, :], 0),
                                in_=lv(src.t[z * 8 + 7:z * 8 + 8, :], 1))
                    else:
                        nc.sync.dma_start(out=lv(g.t[0:128:8, :], 0),
                                          in_=lv(src.t[7:128:8, :], 1))
            return g

        # ---------- mass term: O = (MASS+4) * F ----------
        V.tensor_scalar_mul(O.all_ap(1, 5), F.all_ap(1, 5), float(MASS + 4.0))

        # ---------- Wilson hops ----------
        with tc.tile_pool(name="wp", bufs=1) as wpool:
            for k in range(4):
                ax = k  # lattice axis
                Uk = load_dir(k, "glA")
                # ---- forward hop ----
                rng = (0, 4) if k == 0 else (1, 5)
                s0, s1 = rng
                h = Grp(mk(wpool, 12 * (s1 - s0) * SL, "h"),
                        6, s0, s1 - s0)
                # h[c,j] = psi[c,j] + g(j)*psi[c,p(j)]
                for j in range(2):
                    pj, gj = GCOL[k][j]
                    for r in range(2):
                        pr, psgn = (_phase_parts(gj)[0:2] if r == 0
                                    else _phase_parts(gj)[2:4])
                        dst = h.gap(j, 2, 3, r, s0, s1)       # c-packed
                        a0 = F.gap(j, 4, 3, r, s0, s1)        # psi[c,j].r
                        a1 = F.gap(pj, 4, 3, pr, s0, s1)
                        stt(dst, a1, psgn, a0, AL.add)
                phi = Grp(mk(wpool, 12 * (s1 - s0) * SL, "phi"),
                          6, s0, s1 - s0)
                cmm(phi, Uk, h, s0, s1, adag=True, nj=2)
                # shift (-1 along ax) then reconstruct into O
                if k == 0:
                    psh, dt = phi, -1
                else:
                    psh, dt = mat_shift(phi, ax, -1, 1, 5, "psh", wpool), 0
                # out[:, s'] += -0.5 * rho(s') * psh[:, q(s')]
                rec = [(0, 1.0), (1, 1.0), PROJ_F[k][0], PROJ_F[k][1]]
                for sp in range(4):
                    q, rho = rec[sp]
                    for r in range(2):
                        pr, psgn = (_phase_parts(rho)[0:2] if r == 0
                                    else _phase_parts(rho)[2:4])
                        dst = O.gap(sp, 4, 3, r, 1, 5)
                        src_ = psh.gap(q, 2, 3, pr, 1 + dt, 5 + dt)
                        stt(dst, src_, -0.5 * psgn, dst, AL.add)
                # ---- backward hop ----
                rng = (2, 6) if k == 0 else (1, 5)
                s0, s1 = rng
                hb = Grp(mk(wpool, 12 * (s1 - s0) * SL, "h"),
                         6, s0, s1 - s0)
                # hb[c,j] = g(j)*psi[c,p(j)] - psi[c,j]
                for j in range(2):
                    pj, gj = GCOL[k][j]
                    for r in range(2):
                        pr, psgn = (_phase_parts(gj)[0:2] if r == 0
                                    else _phase_parts(gj)[2:4])
                        dst = hb.gap(j, 2, 3, r, s0, s1)
                        a1 = F.gap(pj, 4, 3, pr, s0, s1)
                        a0 = F.gap(j, 4, 3, r, s0, s1)
                        # dst = psgn*a1 - a0  -> stt: (a0 * -1) + ...? need scaled a1.
                        # use: dst = (a1*psgn) + (-a0): two-step via subtract:
                        # dst = (a1 * psgn) sub? op1 options: use subtract_rev?
                        # simplest: dst = (a1*psgn) + a0*(-1): do STT then sub.
                        stt(dst, a1, psgn, a0, AL.subtract)
                if k == 0:
                    hs, dt = hb, +1
                else:
                    hs, dt = mat_shift(hb, ax, +1, 1, 5, "psh", wpool), 0
                gm = Grp(mk(wpool, 12 * 4 * SL, "phi"),
                         6, 1, 4)
                cmm(gm, Uk, hs, 1, 5, nj=2, bdt=dt)
                rec = [(0, 1.0), (1, 1.0), PROJ_B[k][0], PROJ_B[k][1]]
                for sp in range(4):
                    q, rho = rec[sp]
                    for r in range(2):
                        pr, psgn = (_phase_parts(rho)[0:2] if r == 0
                                    else _phase_parts(rho)[2:4])
                        dst = O.gap(sp, 4, 3, r, 1, 5)
                        src_ = gm.gap(q, 2, 3, pr, 1, 5)
                        stt(dst, src_, 0.5 * psgn, dst, AL.add)

        # ---------- clover planes ----------
        with tc.tile_pool(name="cp", bufs=1) as cpool:
            def ctile(tag, nent, s0, s1):
                return Grp(mk(cpool, nent * 2 * (s1 - s0) * SL, tag), nent, s0, s1 - s0)

            for d1 in range(1, 5):
                for d2 in range(d1 + 1, 5):
                    a1, a2 = d1 - 1, d2 - 1
                    tpl = (d1 == 1)
                    sA, eA = (0, 5) if tpl else (1, 5)
                    U1 = load_dir(d1 - 1, "glA")
                    U2 = load_dir(d2 - 1, "glB")
                    # shifted links
                    if tpl:
                        U2m, u2dt = U2, +1       # U_d2(x+e_T): column view
                    else:
                        U2m, u2dt = mat_shift(U2, a1, +1, sA, eA, "lnk1", cpool), 0
                    U1n = mat_shift(U1, a2, +1, sA, eA, "lnk2", cpool)
                    A = ctile("pA", 9, sA, eA)
                    cmm(A, U1, U2m, sA, eA, bdt=u2dt)
                    B = ctile("pB", 9, sA, eA)
                    cmm(B, U2, U1n, sA, eA)
                    Q = ctile("pQ", 9, 1, 5)
                    cmm(Q, B, A, 1, 5, bdag=True)            # L4
                    sL1, eL1 = (0, 4) if tpl else (1, 5)
                    L1 = ctile("pL", 9, sL1, eL1)
                    cmm(L1, A, B, sL1, eL1, adag=True)
                    # Q += shift_{-a1,-a2}(L1)
                    if tpl:
                        L1s = mat_shift(L1, a2, -1, 0, 4, "pLs", cpool)
                        V.tensor_add(Q.all_ap(1, 5), Q.all_ap(1, 5),
                                     L1s.all_ap(0, 4))
                    else:
                        L1s = mat_shift(L1, a1, -1, 1, 5, "pLs", cpool)
                        L1ss = mat_shift(L1s, a2, -1, 1, 5, "pLss", cpool)
                        V.tensor_add(Q.all_ap(1, 5), Q.all_ap(1, 5),
                                     L1ss.all_ap(1, 5))
                    D = ctile("pA", 9, sA, eA)
                    cmm(D, U1n, U2m, sA, eA, bdag=True, bdt=u2dt)
                    E = ctile("pB", 9, sA, eA)
                    cmm(E, U2, U1, sA, eA, adag=True)
                    L2 = ctile("pL", 9, 1, 5)
                    cmm(L2, D, E, 1, 5, bdag=True)
                    L2s = mat_shift(L2, a2, -1, 1, 5, "pLs", cpool)
                    V.tensor_add(Q.all_ap(1, 5), Q.all_ap(1, 5),
                                 L2s.all_ap(1, 5))
                    sL3, eL3 = (0, 4) if tpl else (1, 5)
                    L3 = ctile("pL", 9, sL3, eL3)
                    cmm(L3, E, D, sL3, eL3, adag=True)
                    if tpl:
                        V.tensor_add(Q.all_ap(1, 5), Q.all_ap(1, 5),
                                     L3.all_ap(0, 4))
                    else:
                        L3s = mat_shift(L3, a1, -1, 1, 5, "pLs", cpool)
                        V.tensor_add(Q.all_ap(1, 5), Q.all_ap(1, 5),
                                     L3s.all_ap(1, 5))
                    # ---- G9 = Q - Q^dag ----
                    G9 = ctile("pG9", 9, 1, 5)
                    qv = Q.t.rearrange("p (i j r w) -> p i j r w",
                                       i=3, j=3, r=2, w=Q.W)
                    qT = qv.transpose([0, 2, 1, 3, 4])
                    gv = G9.t.rearrange("p (i j r w) -> p i j r w",
                                        i=3, j=3, r=2, w=G9.W)
                    V.tensor_sub(gv[:, :, :, 0, :], qv[:, :, :, 0, :],
                                 qT[:, :, :, 0, :])
                    V.tensor_add(gv[:, :, :, 1, :], qv[:, :, :, 1, :],
                                 qT[:, :, :, 1, :])
                    # ---- apply: O -= 1/16 * G9 * (psi sigma) ----
                    w4 = 4 * SL
                    tgv = tview2(mtA, 3, 3, w4)
                    tg1, tg2 = tgv[:, 0], tgv[:, 1]
                    sv4 = sview2(msA, 3, w4)[:, 0]
                    for sp in range(4):
                        wrow, phi_ph = SIGCOL[a1][a2][sp]
                        vr_p, vr_s, vi_p, vi_s = _phase_parts(phi_ph)
                        # v.re plane = vr_s * psi[j, wrow].(vr_p)
                        for outr in range(2):
                            # out.re += sum_j gr*v.re - gi*v.im
                            # out.im += sum_j gr*v.im + gi*v.re
                            if outr == 0:
                                pa, sa_ = vr_p, vr_s
                                pb, sb_ = vi_p, -vi_s
                            else:
                                pa, sa_ = vi_p, vi_s
                                pb, sb_ = vr_p, vr_s
                            # psi plane for color j: entry j*4+wrow, part pa;
                            # broadcast over i (t layout [p, i, j, w])
                            psA = F.gap(wrow, 4, 3, pa, 1, 5)   # [p,3(j),w]
                            psB = F.gap(wrow, 4, 3, pb, 1, 5)
                            psA4 = psA.unsqueeze(1).broadcast_to((128, 3, 3, w4))
                            psB4 = psB.unsqueeze(1).broadcast_to((128, 3, 3, w4))
                            V.tensor_mul(tg1, gv[:, :, :, 0, :], psA4)
                            V.tensor_mul(tg2, gv[:, :, :, 1, :], psB4)
                            # d = sa_*t1 + sb_*t2 ; reduce over j ; O += -1/16*d
                            if sa_ * sb_ > 0:
                                V.tensor_add(tg1, tg1, tg2)
                            else:
                                V.tensor_sub(tg1, tg1, tg2)
                            V.tensor_add(sv4, tg1[:, :, 0, :], tg1[:, :, 1, :])
                            V.tensor_add(sv4, sv4, tg1[:, :, 2, :])
                            dst = O.gap(sp, 4, 3, outr, 1, 5)
                            stt(dst, sv4, -(1.0 / 16.0) * sa_, dst, AL.add)

        nc.sync.dma_start(out=out_d[:, :], in_=O.t[:, :])
        spool_cm.__exit__(None, None, None)
        gpool_cm.__exit__(None, None, None)
        main_cm.__exit__(None, None, None)

    nc.compile()
    return nc


# ----------------------------------------------------------------------
# host entry
# ----------------------------------------------------------------------
_CACHE = {}

def _get_nc():
    if "nc" not in _CACHE:
        _CACHE["nc"] = build_program()
    return _CACHE["nc"]


def kernel(field_re, field_im, gauge_re, gauge_im):
    from concourse.bass_utils import run_bass_kernel_spmd
    nc = _get_nc()
    in_maps = [_host_inputs(field_re, field_im, gauge_re, gauge_im, c)
               for c in range(NCORES)]
    br = run_bass_kernel_spmd(nc, in_maps, list(range(NCORES)))
    out = np.empty((T_, Z_, Y_, X_, 3, 4, 2), np.float32)
    for c in range(NCORES):
        out[c * TLOC:(c + 1) * TLOC] = _host_output(br.results[c]["outp"])
    return out
